# revision 1
# baseline (speedup 1.0000x reference)
"""GQA forward (b=2, s=2048, H=32 q heads, 8 kv heads, d=64) on 8 TRN2 cores.

Sharding: core k owns query heads 4k..4k+3 and kv head k. GQA group
structure makes attention fully local per core (q heads 4k..4k+3 attend
only to kv head k). x is replicated; W columns are sharded; outputs are
column-concatenated.

Per-core kernel (Tile framework):
  - x.T tiles produced via PE transpose, evicted by DVE.
  - Projections in natural layout: QKV[s,384] = xT_chunk.T @ W_chunk
    accumulated over 16 k-chunks (x.T stationary, W moving).
  - RoPE on DVE with free-dim stride-2 views (partition strides are
    illegal), fused with the PSUM->SBUF eviction; V columns pass through
    straight into the [V|1] resident (already [kv, d] natural layout).
  - Q/K flipped to [d, s] via PE transposes.
  - Attention in transposed layout: S.T[kv,q] = K @ Q.T per 128-kv block,
    exp on ACT (scale=1/8 folded in), causal handled by skipping blocks
    above the diagonal + triangular predicated masks on diagonal blocks,
    ctx.T[65,q] = [V|1].T @ P.T accumulated in PSUM (row 64 = softmax sums).
  - Finalize: PE-transpose ctx.T back to [q,d], normalize by row sums, DMA.
Matmuls run as float32r (fp32 storage, full-rate PE mode).
"""

import numpy as np
from contextlib import ExitStack

import concourse.bass as bass
import concourse.bacc as bacc
import concourse.mybir as mybir
from concourse import tile
from concourse.bass_utils import run_bass_kernel_spmd

F32 = mybir.dt.float32
F32R = mybir.dt.float32r
U8 = mybir.dt.uint8
MUL = mybir.AluOpType.mult
ADD = mybir.AluOpType.add

B = 2
S = 2048
DIN = 2048
D = 64              # head dim
HPC = 4             # query heads per core
NCORES = 8
WCOLS = 4 * D + D + D  # 256 q cols + 64 k + 64 v = 384
RC = 320            # roped columns (4 q heads + k head)
ST = 512            # s-tile (rows per outer step)
NST = B * S // ST   # 8 s-tiles
NCH = DIN // 128    # 16 k-chunks
NKV = S // 128      # kv tiles per batch
NEG = -30000.0      # pre-scale mask fill; exp(NEG/8) == 0 in f32


def build_bass():
    nc = bacc.Bacc(None, target_bir_lowering=False)
    x_d = nc.declare_dram_parameter("x", [B * S, DIN], F32, isOutput=False)
    w_d = nc.declare_dram_parameter("w", [DIN, WCOLS], F32, isOutput=False)
    cos_d = nc.declare_dram_parameter("cosn", [S, RC], F32, isOutput=False)
    sin_d = nc.declare_dram_parameter("sinn", [S, RC], F32, isOutput=False)
    mask_d = nc.declare_dram_parameter("mask", [128, 128], U8, isOutput=False)
    id_d = nc.declare_dram_parameter("ident", [128, 128], F32, isOutput=False)
    out_d = nc.declare_dram_parameter("out", [B * S, HPC * D], F32, isOutput=True)

    with ExitStack() as ctx:
        tc = ctx.enter_context(tile.TileContext(nc))
        const = ctx.enter_context(tc.tile_pool(name="const", bufs=1))
        resid = ctx.enter_context(tc.tile_pool(name="resid", bufs=1))
        xa_p = ctx.enter_context(tc.tile_pool(name="xa", bufs=2))
        xt_p = ctx.enter_context(tc.tile_pool(name="xt", bufs=2))
        tab_p = ctx.enter_context(tc.tile_pool(name="tab", bufs=3))
        qn_p = ctx.enter_context(tc.tile_pool(name="qn", bufs=3))
        qt_p = ctx.enter_context(tc.tile_pool(name="qt", bufs=4))
        p_p = ctx.enter_context(tc.tile_pool(name="p", bufs=3))
        cx_p = ctx.enter_context(tc.tile_pool(name="cx", bufs=2))
        o_p = ctx.enter_context(tc.tile_pool(name="o", bufs=3))
        rv_p = ctx.enter_context(tc.tile_pool(name="rv", bufs=4))
        tp_ps = ctx.enter_context(tc.tile_pool(name="tp_ps", bufs=2, space="PSUM"))
        pr_ps = ctx.enter_context(tc.tile_pool(name="pr_ps", bufs=2, space="PSUM"))
        sc_ps = ctx.enter_context(tc.tile_pool(name="sc_ps", bufs=2, space="PSUM"))
        cx_ps = ctx.enter_context(tc.tile_pool(name="cx_ps", bufs=1, space="PSUM"))
        fi_ps = ctx.enter_context(tc.tile_pool(name="fi_ps", bufs=1, space="PSUM"))

        # constants
        w_sb = const.tile([128, NCH, WCOLS], F32R)
        nc.sync.dma_start(
            out=w_sb[:],
            in_=w_d.rearrange("(c p) n -> p c n", p=128).bitcast(F32R))
        mask_sb = const.tile([128, 128], U8)
        nc.sync.dma_start(out=mask_sb[:], in_=mask_d[:])
        ident = const.tile([128, 128], F32R)
        nc.sync.dma_start(out=ident[:], in_=id_d[:].bitcast(F32R))
        neg_sb = const.tile([128, 128], F32)
        nc.vector.memset(neg_sb[:], NEG)
        one_sb = const.tile([128, 1], F32)
        nc.vector.memset(one_sb[:], 1.0)
        zero_sb = const.tile([128, 63], F32)
        nc.vector.memset(zero_sb[:], 0.0)

        # rows 0-63: K.T (RoPE'd); rows 64-127: duplicate copy so that the
        # scores matmul lhsT can match either base partition of the Q halves
        kt_res = resid.tile([128, B * S], F32R)
        vp_res = resid.tile([128, B * NKV, 128], F32R)  # [V|1|0pad] kv-tiles
        for slot in range(B * NKV):
            nc.vector.tensor_copy(vp_res[:, slot, 64:65], one_sb[:])
            nc.vector.tensor_copy(vp_res[:, slot, 65:128], zero_sb[:])

        for st in range(NST):
            b, sti = divmod(st, 4)

            # ---- x rows -> x.T tiles ----
            xt = xt_p.tile([128, NCH, ST], F32R)
            for pt in range(4):
                xa = xa_p.tile([128, DIN], F32R, tag="xa")
                nc.sync.dma_start(
                    out=xa[:],
                    in_=x_d[st * ST + pt * 128:
                            st * ST + (pt + 1) * 128, :].bitcast(F32R))
                for c in range(NCH):
                    tp = tp_ps.tile([128, 128], F32R, tag="tp")
                    nc.tensor.transpose(tp[:], xa[:, c * 128:(c + 1) * 128],
                                        ident[:])
                    nc.vector.tensor_copy(
                        xt[:, c, pt * 128:(pt + 1) * 128], tp[:])

            # ---- projections (natural layout) + RoPE + transposes ----
            qta = qt_p.tile([128, ST], F32R, tag="qta")   # heads 0,1 as [d,s]
            qtb = qt_p.tile([128, ST], F32R, tag="qtb")   # heads 2,3 as [d,s]
            for pt in range(4):
                t = sti * 4 + pt  # within-batch 128-row block index
                pp = pr_ps.tile([128, WCOLS], F32, tag="pp")
                for c in range(NCH):
                    nc.tensor.matmul(
                        pp[:], xt[:, c, pt * 128:(pt + 1) * 128],
                        w_sb[:, c, :], start=(c == 0), stop=(c == NCH - 1))
                ctab = tab_p.tile([128, RC], F32, tag="ctab")
                nc.sync.dma_start(out=ctab[:],
                                  in_=cos_d[t * 128:(t + 1) * 128, :])
                stab = tab_p.tile([128, RC], F32, tag="stab")
                nc.sync.dma_start(out=stab[:],
                                  in_=sin_d[t * 128:(t + 1) * 128, :])
                qn = qn_p.tile([128, WCOLS], F32R, tag="qn")
                ts = qn_p.tile([128, RC], F32, tag="ts")
                # even cols: qe*c - qo*s ; odd cols: qo*c + qe*s
                nc.vector.scalar_tensor_tensor(
                    ts[:, 0:RC:2], pp[:, 1:RC:2], -1.0, stab[:, 0:RC:2],
                    MUL, MUL)
                nc.vector.tensor_tensor(
                    ts[:, 1:RC:2], pp[:, 0:RC:2], stab[:, 1:RC:2], MUL)
                nc.vector.tensor_tensor(qn[:, 0:RC], pp[:, 0:RC], ctab[:], MUL)
                nc.vector.tensor_tensor(qn[:, 0:RC], qn[:, 0:RC], ts[:], ADD)
                # V columns: straight into the [V|1] resident (natural [kv,d])
                nc.vector.tensor_copy(
                    vp_res[:, b * NKV + t, 0:64], pp[:, RC:WCOLS])
                # also land V in qn so the padded K-flip transpose below reads
                # initialized data (its V rows are discarded)
                nc.vector.tensor_copy(qn[:, RC:WCOLS], pp[:, RC:WCOLS])
                # flip Q/K to [d, s]
                for cb in range(2):
                    tp = tp_ps.tile([128, 128], F32R, tag="tp")
                    nc.tensor.transpose(
                        tp[:], qn[:, cb * 128:(cb + 1) * 128], ident[:])
                    dst = qta if cb == 0 else qtb
                    nc.vector.tensor_copy(
                        dst[:, pt * 128:(pt + 1) * 128], tp[:])
                tp = tp_ps.tile([128, 128], F32R, tag="tp")
                nc.tensor.transpose(tp[:], qn[:, 256:384], ident[:])
                nc.vector.tensor_copy(
                    kt_res[0:64, st * ST + pt * 128:st * ST + (pt + 1) * 128],
                    tp[0:64, :])
            nc.sync.dma_start(
                out=kt_res[64:128, st * ST:(st + 1) * ST],
                in_=kt_res[0:64, st * ST:(st + 1) * ST])

            # ---- attention for the 4 heads of this q-tile ----
            js = [4 * sti] + list(range(4 * sti)) + \
                 [4 * sti + 1, 4 * sti + 2, 4 * sti + 3]
            for h in range(HPC):
                p0 = (h % 2) * 64
                qh = (qta if h < 2 else qtb)[p0:p0 + 64, :]
                cxt = cx_ps.tile([128, ST], F32, tag="cxt")
                for idx, j in enumerate(js):
                    off = 128 * j - 512 * sti
                    if j <= 4 * sti:
                        w0 = 0
                    elif off == 128:
                        w0 = 128
                    else:
                        w0 = 256
                    sc = sc_ps.tile([128, ST], F32, tag="sc")
                    nc.tensor.matmul(
                        sc[:, w0:ST],
                        kt_res[p0:p0 + 64, b * S + j * 128:b * S + (j + 1) * 128],
                        qh[:, w0:ST], start=True, stop=True)
                    if j == 4 * sti + 3:
                        nc.vector.tensor_copy(sc[:, 256:384], neg_sb[:])
                    if j >= 4 * sti:
                        nc.vector.copy_predicated(
                            sc[:, off:off + 128], mask_sb[:], neg_sb[:])
                    psb = p_p.tile([128, ST], F32R, tag="psb")
                    nc.scalar.activation(
                        psb[:, w0:ST], sc[:, w0:ST],
                        mybir.ActivationFunctionType.Exp, scale=0.125)
                    nc.tensor.matmul(
                        cxt[:, w0:ST], vp_res[:, b * NKV + j, :],
                        psb[:, w0:ST],
                        start=(idx == 0), stop=(idx == len(js) - 1))
                cxs = cx_p.tile([65, ST], F32R, tag="cxs")
                nc.vector.tensor_copy(cxs[:], cxt[0:65, :])
                for qq in range(4):
                    fi = fi_ps.tile([128, 66], F32R, tag="fi")
                    nc.tensor.transpose(fi[:], cxs[:, qq * 128:(qq + 1) * 128],
                                        ident[0:65, 0:66])
                    rv = rv_p.tile([128, 1], F32, tag="rv")
                    nc.vector.reciprocal(rv[:], fi[:, 64:65])
                    ob = o_p.tile([128, 64], F32, tag="ob")
                    nc.vector.tensor_scalar_mul(ob[:], fi[:, 0:64], rv[:])
                    nc.sync.dma_start(
                        out=out_d[st * ST + qq * 128:st * ST + (qq + 1) * 128,
                                  h * 64:(h + 1) * 64],
                        in_=ob[:])
    return nc


_NC_CACHE = None


def _host_consts():
    i = np.arange(0, D, 2, dtype=np.float64) / D          # 32 pair exponents
    freqs = 1.0 / (10000.0 ** i)                           # (32,)
    ang = np.arange(S, dtype=np.float64)[:, None] * freqs[None, :]  # (S, 32)
    cos = np.cos(ang).astype(np.float32)                   # (S, 32)
    sin = np.sin(ang).astype(np.float32)
    dcol = (np.arange(RC) % D) // 2                        # (320,) pair idx
    cosn = np.ascontiguousarray(cos[:, dcol])              # (S, 320)
    sinn = np.ascontiguousarray(sin[:, dcol])
    kv, qq = np.meshgrid(np.arange(128), np.arange(128), indexing="ij")
    maskinv = (kv > qq).astype(np.uint8)                   # 1 = forbidden
    ident = np.eye(128, dtype=np.float32)
    return cosn, sinn, maskinv, ident


def _in_maps(x, Wq, Wk, Wv):
    x = np.asarray(x, dtype=np.float32).reshape(B * S, DIN)
    Wq = np.asarray(Wq, dtype=np.float32)
    Wk = np.asarray(Wk, dtype=np.float32)
    Wv = np.asarray(Wv, dtype=np.float32)
    cosn, sinn, maskinv, ident = _host_consts()

    in_maps = []
    for k in range(NCORES):
        w_all = np.hstack([
            Wq[:, k * 256:(k + 1) * 256],
            Wk[:, k * 64:(k + 1) * 64],
            Wv[:, k * 64:(k + 1) * 64],
        ]).astype(np.float32)
        in_maps.append({
            "x": x, "w": np.ascontiguousarray(w_all),
            "cosn": cosn, "sinn": sinn, "mask": maskinv, "ident": ident,
        })
    return in_maps


def _run(in_maps, **kwargs):
    global _NC_CACHE
    if _NC_CACHE is None:
        _NC_CACHE = build_bass()
        _NC_CACHE.finalize()
    return run_bass_kernel_spmd(_NC_CACHE, in_maps, list(range(NCORES)),
                                **kwargs)


def kernel(x, Wq, Wk, Wv):
    res = _run(_in_maps(x, Wq, Wk, Wv))
    out = np.concatenate([res.results[k]["out"] for k in range(NCORES)], axis=1)
    return out.reshape(B, S, 32 * D)



# revision 4
# speedup vs baseline: 1.4232x; 1.4232x over previous
"""GQA forward (b=2, s=2048, H=32 q heads, 8 kv heads, d=64) on 8 TRN2 cores.

Sharding: core k owns query heads 4k..4k+3 and kv head k. GQA group
structure makes attention fully local per core (q heads 4k..4k+3 attend
only to kv head k). x is replicated; W columns are sharded; outputs are
column-concatenated.

v2 layout (all matmul operands bf16; HW fp32r streams at 2-4 cyc/row while
bf16 streams at 1, and bf16 halves DMA bytes):
  - x is transposed + bf16-cast on the HOST; x.T tiles DMA straight into
    SBUF (kills the 512 PE transposes + 512 DVE evictions of v1).
  - Projections in natural layout: QKV[s,384] accumulated over 16 k-chunks.
  - RoPE on DVE (stride-2 free-dim views), output bf16.
  - Q/K flips via the DMA-transpose XBAR (16x128 tiles, bf16) instead of
    PE transposes: qn[:,0:128]->qta, qn[:,128:256]->qtb, qn[:,256:384]->
    kt_res rows 0:128 (V.T rows then overwritten by the kt dup DMA).
  - Attention in transposed layout: S.T[kv,q] = K @ Q.T per 128-kv block,
    two kv blocks share one PSUM tile so exp batches 2 strips per ACT
    instruction; causal via triangular predicated masks on diagonal
    blocks; ctx.T[80,q] = [V|1|0pad].T @ P.T accumulated in PSUM (row 64 =
    softmax sums, rows 65:80 zero pad so downstream reads are initialized).
  - Finalize: evict ctx.T to bf16 SBUF, DMA-transpose back to [q,80],
    normalize with a per-partition divide on GpSimd (Pool is otherwise
    idle), assemble [128,4,256] f32 per s-tile, one output DMA per s-tile.
"""

import numpy as np
from contextlib import ExitStack

import ml_dtypes

import concourse.bass as bass
import concourse.bacc as bacc
import concourse.mybir as mybir
from concourse import tile
from concourse.bass_utils import run_bass_kernel_spmd

F32 = mybir.dt.float32
BF16 = mybir.dt.bfloat16
U8 = mybir.dt.uint8
MUL = mybir.AluOpType.mult
ADD = mybir.AluOpType.add
DIV = mybir.AluOpType.divide
EXP = mybir.ActivationFunctionType.Exp

B = 2
S = 2048
DIN = 2048
D = 64              # head dim
HPC = 4             # query heads per core
NCORES = 8
WCOLS = 4 * D + D + D  # 256 q cols + 64 k + 64 v = 384
RC = 320            # roped columns (4 q heads + k head)
ST = 512            # s-tile (rows per outer step)
NST = B * S // ST   # 8 s-tiles
NCH = DIN // 128    # 16 k-chunks
NKV = S // 128      # kv tiles per batch
NEG = -30000.0      # pre-scale mask fill; exp(NEG/8) == 0 in f32


def build_bass():
    nc = bacc.Bacc(None, target_bir_lowering=False)
    xt_d = nc.declare_dram_parameter("xt", [DIN, B * S], BF16, isOutput=False)
    w_d = nc.declare_dram_parameter("w", [DIN, WCOLS], BF16, isOutput=False)
    cos_d = nc.declare_dram_parameter("cosn", [S, RC], F32, isOutput=False)
    sin_d = nc.declare_dram_parameter("sinn", [S, RC], F32, isOutput=False)
    mask_d = nc.declare_dram_parameter("mask", [128, 128], U8, isOutput=False)
    out_d = nc.declare_dram_parameter("out", [B * S, HPC * D], F32, isOutput=True)

    with ExitStack() as ctx:
        tc = ctx.enter_context(tile.TileContext(nc))
        const = ctx.enter_context(tc.tile_pool(name="const", bufs=1))
        resid = ctx.enter_context(tc.tile_pool(name="resid", bufs=1))
        xt_p = ctx.enter_context(tc.tile_pool(name="xt", bufs=2))
        tab_p = ctx.enter_context(tc.tile_pool(name="tab", bufs=2))
        qn_p = ctx.enter_context(tc.tile_pool(name="qn", bufs=2))
        qt_p = ctx.enter_context(tc.tile_pool(name="qt", bufs=2))
        p_p = ctx.enter_context(tc.tile_pool(name="p", bufs=3))
        cxs_p = ctx.enter_context(tc.tile_pool(name="cxs", bufs=2))
        fo_p = ctx.enter_context(tc.tile_pool(name="fo", bufs=3))
        ob_p = ctx.enter_context(tc.tile_pool(name="ob", bufs=2))
        pr_ps = ctx.enter_context(tc.tile_pool(name="pr_ps", bufs=2, space="PSUM"))
        sc_ps = ctx.enter_context(tc.tile_pool(name="sc_ps", bufs=2, space="PSUM"))
        cx_ps = ctx.enter_context(tc.tile_pool(name="cx_ps", bufs=2, space="PSUM"))

        # constants / residents
        w_sb = const.tile([128, NCH, WCOLS], BF16)
        nc.sync.dma_start(
            out=w_sb[:], in_=w_d.rearrange("(c p) n -> p c n", p=128))
        mask_sb = const.tile([128, 128], U8)
        nc.sync.dma_start(out=mask_sb[:], in_=mask_d[:])
        neg_sb = const.tile([128, 128], F32)
        nc.vector.memset(neg_sb[:], NEG)

        # rows 0-63: K.T (RoPE'd); rows 64-127: duplicate copy so the scores
        # matmul lhsT can match either base partition of the Q halves
        kt_res = resid.tile([128, B * S], BF16)
        # [V | 1 | 0-pad] per kv tile: col 64 = ones (softmax sums land in
        # ctx.T row 64), cols 65:80 zero so ctx.T rows 65:80 read initialized
        vp_res = resid.tile([128, B * NKV, 80], BF16)
        nc.vector.memset(vp_res[:, :, 64:65], 1.0)
        nc.vector.memset(vp_res[:, :, 65:80], 0.0)

        xt_view = xt_d.rearrange("(c p) s -> p c s", p=128)

        for st in range(NST):
            b, sti = divmod(st, 4)

            # ---- x.T tiles straight from DRAM (host pre-transposed) ----
            xt = xt_p.tile([128, NCH, ST], BF16)
            nc.sync.dma_start(
                out=xt[:], in_=xt_view[:, :, st * ST:(st + 1) * ST])

            # ---- projections (natural layout) + RoPE + DMA-transpose ----
            qta = qt_p.tile([128, ST], BF16, tag="qta")   # heads 0,1 as [d,s]
            qtb = qt_p.tile([128, ST], BF16, tag="qtb")   # heads 2,3 as [d,s]
            for pt in range(4):
                t = sti * 4 + pt  # within-batch 128-row block index
                pp = pr_ps.tile([128, WCOLS], F32, tag="pp")
                for c in range(NCH):
                    nc.tensor.matmul(
                        pp[:], xt[:, c, pt * 128:(pt + 1) * 128],
                        w_sb[:, c, :], start=(c == 0), stop=(c == NCH - 1))
                ctab = tab_p.tile([128, RC], F32, tag="ctab")
                nc.sync.dma_start(out=ctab[:],
                                  in_=cos_d[t * 128:(t + 1) * 128, :])
                stab = tab_p.tile([128, RC], F32, tag="stab")
                nc.sync.dma_start(out=stab[:],
                                  in_=sin_d[t * 128:(t + 1) * 128, :])
                qn = qn_p.tile([128, WCOLS], BF16, tag="qn")
                ts = qn_p.tile([128, RC], BF16, tag="ts")
                # even cols: qe*c - qo*s ; odd cols: qo*c + qe*s
                nc.vector.scalar_tensor_tensor(
                    ts[:, 0:RC:2], pp[:, 1:RC:2], -1.0, stab[:, 0:RC:2],
                    MUL, MUL)
                nc.vector.tensor_tensor(
                    ts[:, 1:RC:2], pp[:, 0:RC:2], stab[:, 1:RC:2], MUL)
                nc.vector.tensor_tensor(qn[:, 0:RC], pp[:, 0:RC], ctab[:], MUL)
                nc.vector.tensor_tensor(qn[:, 0:RC], qn[:, 0:RC], ts[:], ADD)
                # V columns: into the [V|1|0] resident (natural [kv, d])
                nc.vector.tensor_copy(
                    vp_res[:, b * NKV + t, 0:64], pp[:, RC:WCOLS])
                # and into qn so the K|V DMA-transpose reads initialized data
                nc.vector.tensor_copy(qn[:, RC:WCOLS], pp[:, RC:WCOLS])
                # flips via DMA-transpose XBAR
                nc.sync.dma_start(
                    out=qta[:, pt * 128:(pt + 1) * 128],
                    in_=qn[:, 0:128], transpose=True)
                nc.sync.dma_start(
                    out=qtb[:, pt * 128:(pt + 1) * 128],
                    in_=qn[:, 128:256], transpose=True)
                # rows 0:64 = K.T, rows 64:128 = V.T (overwritten by dup)
                nc.sync.dma_start(
                    out=kt_res[:, st * ST + pt * 128:st * ST + (pt + 1) * 128],
                    in_=qn[:, 256:384], transpose=True)
            nc.sync.dma_start(
                out=kt_res[64:128, st * ST:(st + 1) * ST],
                in_=kt_res[0:64, st * ST:(st + 1) * ST])

            ob = ob_p.tile([128, 4, HPC * D], F32)

            # ---- attention for the 4 heads of this q-tile ----
            for h in range(HPC):
                p0 = (h % 2) * 64
                qh = (qta if h < 2 else qtb)[p0:p0 + 64, :]

                def kt(j):
                    return kt_res[p0:p0 + 64,
                                  b * S + j * 128:b * S + (j + 1) * 128]

                def vp(j):
                    return vp_res[:, b * NKV + j, :]

                cxt = cx_ps.tile([128, ST], F32, tag="cxt")
                first = True
                # full sub-diagonal blocks, two kv blocks per PSUM tile
                for jp in range(0, 4 * sti, 2):
                    sc = sc_ps.tile([128, 2, ST], F32, tag="sc")
                    for jj in (0, 1):
                        nc.tensor.matmul(sc[:, jj, :], kt(jp + jj), qh[:, :],
                                         start=True, stop=True)
                    psb = p_p.tile([128, 2, ST], BF16, tag="psb")
                    nc.scalar.activation(psb[:], sc[:], EXP, scale=0.125)
                    for jj in (0, 1):
                        nc.tensor.matmul(
                            cxt[0:80, :], vp(jp + jj), psb[:, jj, :],
                            start=first, stop=False)
                        first = False
                # diagonal strips r=0..3 (kv block 4*sti+r vs q cols
                # 128r:512), packed two per PSUM tile
                j0 = 4 * sti
                scd = sc_ps.tile([128, 2 * ST], F32, tag="sc")
                nc.tensor.matmul(scd[:, 0:512], kt(j0), qh[:, :],
                                 start=True, stop=True)
                nc.tensor.matmul(scd[:, 512:896], kt(j0 + 1), qh[:, 128:512],
                                 start=True, stop=True)
                nc.vector.copy_predicated(scd[:, 0:128], mask_sb[:], neg_sb[:])
                nc.vector.copy_predicated(scd[:, 512:640], mask_sb[:],
                                          neg_sb[:])
                psbd = p_p.tile([128, 2 * ST], BF16, tag="psb")
                nc.scalar.activation(psbd[:, 0:896], scd[:, 0:896], EXP,
                                     scale=0.125)
                nc.tensor.matmul(cxt[0:80, :], vp(j0), psbd[:, 0:512],
                                 start=first, stop=False)
                nc.tensor.matmul(cxt[0:80, 128:512], vp(j0 + 1),
                                 psbd[:, 512:896], start=False, stop=False)

                scd2 = sc_ps.tile([128, 2 * ST], F32, tag="sc")
                nc.tensor.matmul(scd2[:, 0:256], kt(j0 + 2), qh[:, 256:512],
                                 start=True, stop=True)
                nc.tensor.matmul(scd2[:, 256:384], kt(j0 + 3), qh[:, 384:512],
                                 start=True, stop=True)
                nc.vector.copy_predicated(scd2[:, 0:128], mask_sb[:],
                                          neg_sb[:])
                nc.vector.copy_predicated(scd2[:, 256:384], mask_sb[:],
                                          neg_sb[:])
                psbd2 = p_p.tile([128, 2 * ST], BF16, tag="psb")
                nc.scalar.activation(psbd2[:, 0:384], scd2[:, 0:384], EXP,
                                     scale=0.125)
                nc.tensor.matmul(cxt[0:80, 256:512], vp(j0 + 2),
                                 psbd2[:, 0:256], start=False, stop=False)
                nc.tensor.matmul(cxt[0:80, 384:512], vp(j0 + 3),
                                 psbd2[:, 256:384], start=False, stop=True)

                # ---- finalize: ctx.T -> [q, 80] via DMA-transpose ----
                cxs = cxs_p.tile([80, ST], BF16)
                nc.vector.tensor_copy(cxs[:], cxt[0:80, :])
                for qq in range(4):
                    fo = fo_p.tile([128, 80], BF16)
                    nc.sync.dma_start(
                        out=fo[:], in_=cxs[:, qq * 128:(qq + 1) * 128],
                        transpose=True)
                    rv = fo_p.tile([128, 1], F32, tag="rv")
                    nc.vector.reciprocal(rv[:], fo[:, 64:65])
                    nc.vector.tensor_scalar_mul(
                        ob[:, qq, h * 64:(h + 1) * 64], fo[:, 0:64], rv[:])

            nc.sync.dma_start(
                out=out_d[st * ST:(st + 1) * ST, :].rearrange(
                    "(q p) n -> p q n", p=128),
                in_=ob[:])
    return nc


_NC_CACHE = None


def _host_consts():
    i = np.arange(0, D, 2, dtype=np.float64) / D          # 32 pair exponents
    freqs = 1.0 / (10000.0 ** i)                           # (32,)
    ang = np.arange(S, dtype=np.float64)[:, None] * freqs[None, :]  # (S, 32)
    cos = np.cos(ang).astype(np.float32)                   # (S, 32)
    sin = np.sin(ang).astype(np.float32)
    dcol = (np.arange(RC) % D) // 2                        # (320,) pair idx
    cosn = np.ascontiguousarray(cos[:, dcol])              # (S, 320)
    sinn = np.ascontiguousarray(sin[:, dcol])
    kv, qq = np.meshgrid(np.arange(128), np.arange(128), indexing="ij")
    maskinv = (kv > qq).astype(np.uint8)                   # 1 = forbidden
    return cosn, sinn, maskinv


def _in_maps(x, Wq, Wk, Wv):
    x = np.asarray(x, dtype=np.float32).reshape(B * S, DIN)
    xt = np.ascontiguousarray(x.T).astype(ml_dtypes.bfloat16)
    Wq = np.asarray(Wq, dtype=np.float32)
    Wk = np.asarray(Wk, dtype=np.float32)
    Wv = np.asarray(Wv, dtype=np.float32)
    cosn, sinn, maskinv = _host_consts()

    in_maps = []
    for k in range(NCORES):
        w_all = np.hstack([
            Wq[:, k * 256:(k + 1) * 256],
            Wk[:, k * 64:(k + 1) * 64],
            Wv[:, k * 64:(k + 1) * 64],
        ]).astype(ml_dtypes.bfloat16)
        in_maps.append({
            "xt": xt, "w": np.ascontiguousarray(w_all),
            "cosn": cosn, "sinn": sinn, "mask": maskinv,
        })
    return in_maps


def _run(in_maps, **kwargs):
    global _NC_CACHE
    if _NC_CACHE is None:
        _NC_CACHE = build_bass()
        _NC_CACHE.finalize()
    return run_bass_kernel_spmd(_NC_CACHE, in_maps, list(range(NCORES)),
                                **kwargs)


def kernel(x, Wq, Wk, Wv):
    res = _run(_in_maps(x, Wq, Wk, Wv))
    out = np.concatenate([res.results[k]["out"] for k in range(NCORES)], axis=1)
    return out.reshape(B, S, 32 * D)


# revision 12
# speedup vs baseline: 2.2492x; 1.5804x over previous
"""GQA forward (b=2, s=2048, H=32 q heads, 8 kv heads, d=64) on 8 TRN2 cores.

Sharding: core k owns query heads 4k..4k+3 and kv head k. GQA group
structure makes attention fully local per core (q heads 4k..4k+3 attend
only to kv head k). x is replicated; W columns are sharded; outputs are
column-concatenated.

v2 layout (all matmul operands bf16; HW fp32r streams at 2-4 cyc/row while
bf16 streams at 1, and bf16 halves DMA bytes):
  - x is transposed + bf16-cast on the HOST; x.T tiles DMA straight into
    SBUF (kills the 512 PE transposes + 512 DVE evictions of v1).
  - Projections in natural layout: QKV[s,384] accumulated over 16 k-chunks.
  - RoPE on DVE (stride-2 free-dim views), output bf16.
  - Q/K flips via the DMA-transpose XBAR (16x128 tiles, bf16) instead of
    PE transposes: qn[:,0:128]->qta, qn[:,128:256]->qtb, qn[:,256:384]->
    kt_res rows 0:128 (V.T rows then overwritten by the kt dup DMA).
  - Attention in transposed layout: S.T[kv,q] = K @ Q.T per 128-kv block,
    two kv blocks share one PSUM tile so exp batches 2 strips per ACT
    instruction; causal via triangular predicated masks on diagonal
    blocks; ctx.T[80,q] = [V|1|0pad].T @ P.T accumulated in PSUM (row 64 =
    softmax sums, rows 65:80 zero pad so downstream reads are initialized).
  - Finalize: evict ctx.T to bf16 SBUF, DMA-transpose back to [q,80],
    normalize with a per-partition divide on GpSimd (Pool is otherwise
    idle), assemble [128,4,256] f32 per s-tile, one output DMA per s-tile.
"""

import numpy as np
from contextlib import ExitStack

import ml_dtypes

import concourse.bass as bass
import concourse.bacc as bacc
import concourse.mybir as mybir
from concourse import tile
from concourse.bass_utils import run_bass_kernel_spmd

F32 = mybir.dt.float32
BF16 = mybir.dt.bfloat16
U8 = mybir.dt.uint8
MUL = mybir.AluOpType.mult
ADD = mybir.AluOpType.add
DIV = mybir.AluOpType.divide
EXP = mybir.ActivationFunctionType.Exp

B = 2
S = 2048
DIN = 2048
D = 64              # head dim
HPC = 4             # query heads per core
NCORES = 8
WCOLS = 4 * D + D + D  # 256 q cols + 64 k + 64 v = 384
RC = 320            # roped columns (4 q heads + k head)
ST = 512            # s-tile (rows per outer step)
NST = B * S // ST   # 8 s-tiles
NCH = DIN // 128    # 16 k-chunks
NKV = S // 128      # kv tiles per batch
NEG = -30000.0      # pre-scale mask fill; exp(NEG/8) == 0 in f32


def build_bass():
    nc = bacc.Bacc(None, target_bir_lowering=False)
    xt_d = nc.declare_dram_parameter("xt", [DIN, B * S], BF16, isOutput=False)
    w_d = nc.declare_dram_parameter("w", [DIN, WCOLS], BF16, isOutput=False)
    cos_d = nc.declare_dram_parameter("cosn", [S, RC], F32, isOutput=False)
    sin_d = nc.declare_dram_parameter("sinn", [S, RC], F32, isOutput=False)
    mask_d = nc.declare_dram_parameter("mask", [128, 128], U8, isOutput=False)
    id_d = nc.declare_dram_parameter("ident", [128, 128], BF16, isOutput=False)
    out_d = nc.declare_dram_parameter("out", [B * S, HPC * D], F32, isOutput=True)

    with ExitStack() as ctx:
        tc = ctx.enter_context(tile.TileContext(nc))
        const = ctx.enter_context(tc.tile_pool(name="const", bufs=1))
        resid = ctx.enter_context(tc.tile_pool(name="resid", bufs=1))
        xt_p = ctx.enter_context(tc.tile_pool(name="xt", bufs=2))
        tab_p = ctx.enter_context(tc.tile_pool(name="tab", bufs=2))
        qn_p = ctx.enter_context(tc.tile_pool(name="qn", bufs=2))
        qt_p = ctx.enter_context(tc.tile_pool(name="qt", bufs=2))
        p_p = ctx.enter_context(tc.tile_pool(name="p", bufs=4))
        cxs_p = ctx.enter_context(tc.tile_pool(name="cxs", bufs=3))
        fo_p = ctx.enter_context(tc.tile_pool(name="fo", bufs=3))
        ob_p = ctx.enter_context(tc.tile_pool(name="ob", bufs=2))
        pr_ps = ctx.enter_context(tc.tile_pool(name="pr_ps", bufs=2, space="PSUM"))
        sc_ps = ctx.enter_context(tc.tile_pool(name="sc_ps", bufs=2, space="PSUM"))
        cx_ps = ctx.enter_context(tc.tile_pool(name="cx_ps", bufs=1, space="PSUM"))
        fi_ps = ctx.enter_context(tc.tile_pool(name="fi_ps", bufs=1, space="PSUM"))

        # constants / residents
        w_sb = const.tile([128, NCH, WCOLS], BF16)
        nc.sync.dma_start(
            out=w_sb[:], in_=w_d.rearrange("(c p) n -> p c n", p=128))
        mask_sb = const.tile([128, 128], U8)
        nc.sync.dma_start(out=mask_sb[:], in_=mask_d[:])
        ident = const.tile([128, 128], BF16)
        nc.sync.dma_start(out=ident[:], in_=id_d[:])
        neg_sb = const.tile([128, 128], F32)
        nc.vector.memset(neg_sb[:], NEG)

        # rows 0-63: K.T (RoPE'd); rows 64-127: duplicate copy so the scores
        # matmul lhsT can match either base partition of the Q halves
        kt_res = resid.tile([128, B * S], BF16)
        # [V | 1 | 0-pad] per kv tile: col 64 = ones (softmax sums land in
        # ctx.T row 64), cols 65:80 zero so ctx.T rows 65:80 read initialized
        vp_res = resid.tile([128, B * NKV, 80], BF16)
        nc.vector.memset(vp_res[:, :, 64:65], 1.0)
        nc.vector.memset(vp_res[:, :, 65:80], 0.0)

        xt_view = xt_d.rearrange("(c p) s -> p c s", p=128)

        for st in range(NST):
            b, sti = divmod(st, 4)

            # ---- x.T tiles straight from DRAM (host pre-transposed) ----
            xt = xt_p.tile([128, NCH, ST], BF16)
            nc.sync.dma_start(
                out=xt[:], in_=xt_view[:, :, st * ST:(st + 1) * ST])

            # ---- projections (natural layout) + RoPE + DMA-transpose ----
            qta = qt_p.tile([128, ST], BF16, tag="qta")   # heads 0,1 as [d,s]
            qtb = qt_p.tile([128, ST], BF16, tag="qtb")   # heads 2,3 as [d,s]
            ctab = tab_p.tile([128, 4, RC], F32, tag="ctab")
            nc.sync.dma_start(
                out=ctab[:],
                in_=cos_d[sti * ST:(sti + 1) * ST, :].rearrange(
                    "(q p) n -> p q n", p=128))
            stab = tab_p.tile([128, 4, RC], F32, tag="stab")
            nc.sync.dma_start(
                out=stab[:],
                in_=sin_d[sti * ST:(sti + 1) * ST, :].rearrange(
                    "(q p) n -> p q n", p=128))
            for pt in range(4):
                t = sti * 4 + pt  # within-batch 128-row block index
                pp = pr_ps.tile([128, WCOLS], F32, tag="pp")
                for c in range(NCH):
                    nc.tensor.matmul(
                        pp[:], xt[:, c, pt * 128:(pt + 1) * 128],
                        w_sb[:, c, :], start=(c == 0), stop=(c == NCH - 1))
                qn = qn_p.tile([128, WCOLS], BF16, tag="qn")
                ts = qn_p.tile([128, RC], BF16, tag="ts")
                # even cols: qe*c - qo*s ; odd cols: qo*c + qe*s
                nc.vector.scalar_tensor_tensor(
                    ts[:, 0:RC:2], pp[:, 1:RC:2], -1.0, stab[:, pt, 0:RC:2],
                    MUL, MUL)
                nc.vector.tensor_tensor(
                    ts[:, 1:RC:2], pp[:, 0:RC:2], stab[:, pt, 1:RC:2], MUL)
                nc.vector.tensor_tensor(qn[:, 0:RC], pp[:, 0:RC],
                                        ctab[:, pt, :], MUL)
                nc.vector.tensor_tensor(qn[:, 0:RC], qn[:, 0:RC], ts[:], ADD)
                # V columns: into the [V|1|0] resident (natural [kv, d])
                nc.vector.tensor_copy(
                    vp_res[:, b * NKV + t, 0:64], pp[:, RC:WCOLS])
                # and into qn so the K|V DMA-transpose reads initialized data
                nc.vector.tensor_copy(qn[:, RC:WCOLS], pp[:, RC:WCOLS])
                # flips via DMA-transpose XBAR
                nc.sync.dma_start(
                    out=qta[:, pt * 128:(pt + 1) * 128],
                    in_=qn[:, 0:128], transpose=True)
                nc.sync.dma_start(
                    out=qtb[:, pt * 128:(pt + 1) * 128],
                    in_=qn[:, 128:256], transpose=True)
                # rows 0:64 = K.T, rows 64:128 = V.T (overwritten by dup)
                nc.sync.dma_start(
                    out=kt_res[:, st * ST + pt * 128:st * ST + (pt + 1) * 128],
                    in_=qn[:, 256:384], transpose=True)
            nc.sync.dma_start(
                out=kt_res[64:128, st * ST:(st + 1) * ST],
                in_=kt_res[0:64, st * ST:(st + 1) * ST])

            ob = ob_p.tile([128, 4, HPC * D], F32)

            # ---- attention for the 4 heads of this q-tile ----
            for h in range(HPC):
                p0 = (h % 2) * 64
                qh = (qta if h < 2 else qtb)[p0:p0 + 64, :]

                def kt(j):
                    return kt_res[p0:p0 + 64,
                                  b * S + j * 128:b * S + (j + 1) * 128]

                def vp(j):
                    return vp_res[:, b * NKV + j, :]

                cxt = cx_ps.tile([128, ST], F32, tag="cxt")
                first = True
                # full sub-diagonal blocks, two kv blocks per PSUM tile
                for jp in range(0, 4 * sti, 2):
                    sc = sc_ps.tile([128, 2, ST], F32, tag="sc")
                    for jj in (0, 1):
                        nc.tensor.matmul(sc[:, jj, :], kt(jp + jj), qh[:, :],
                                         start=True, stop=True)
                    psb = p_p.tile([128, 2, ST], BF16, tag="psb")
                    nc.scalar.activation(psb[:], sc[:], EXP, scale=0.125)
                    for jj in (0, 1):
                        nc.tensor.matmul(
                            cxt[0:80, :], vp(jp + jj), psb[:, jj, :],
                            start=first, stop=False)
                        first = False
                # diagonal strips r=0..3 (kv block 4*sti+r vs q cols
                # 128r:512), packed two per PSUM tile
                j0 = 4 * sti
                scd = sc_ps.tile([128, 2 * ST], F32, tag="sc")
                nc.tensor.matmul(scd[:, 0:512], kt(j0), qh[:, :],
                                 start=True, stop=True)
                nc.tensor.matmul(scd[:, 512:896], kt(j0 + 1), qh[:, 128:512],
                                 start=True, stop=True)
                nc.vector.copy_predicated(scd[:, 0:128], mask_sb[:], neg_sb[:])
                nc.vector.copy_predicated(scd[:, 512:640], mask_sb[:],
                                          neg_sb[:])
                psbd = p_p.tile([128, 2 * ST], BF16, tag="psb")
                nc.scalar.activation(psbd[:, 0:896], scd[:, 0:896], EXP,
                                     scale=0.125)
                nc.tensor.matmul(cxt[0:80, :], vp(j0), psbd[:, 0:512],
                                 start=first, stop=False)
                nc.tensor.matmul(cxt[0:80, 128:512], vp(j0 + 1),
                                 psbd[:, 512:896], start=False, stop=False)

                scd2 = sc_ps.tile([128, 2 * ST], F32, tag="sc")
                nc.tensor.matmul(scd2[:, 0:256], kt(j0 + 2), qh[:, 256:512],
                                 start=True, stop=True)
                nc.tensor.matmul(scd2[:, 256:384], kt(j0 + 3), qh[:, 384:512],
                                 start=True, stop=True)
                nc.vector.copy_predicated(scd2[:, 0:128], mask_sb[:],
                                          neg_sb[:])
                nc.vector.copy_predicated(scd2[:, 256:384], mask_sb[:],
                                          neg_sb[:])
                psbd2 = p_p.tile([128, 2 * ST], BF16, tag="psb")
                nc.scalar.activation(psbd2[:, 0:384], scd2[:, 0:384], EXP,
                                     scale=0.125)
                nc.tensor.matmul(cxt[0:80, 256:512], vp(j0 + 2),
                                 psbd2[:, 0:256], start=False, stop=False)
                nc.tensor.matmul(cxt[0:80, 384:512], vp(j0 + 3),
                                 psbd2[:, 256:384], start=False, stop=True)

                # ---- finalize: ctx.T -> [q, 80] via PE transpose ----
                cxs = cxs_p.tile([80, ST], BF16)
                nc.vector.tensor_copy(cxs[:], cxt[0:80, :])
                for qq in range(4):
                    fi = fi_ps.tile([128, 80], BF16, tag="fi")
                    nc.tensor.transpose(fi[:], cxs[:, qq * 128:(qq + 1) * 128],
                                        ident[0:80, 0:80])
                    rv = fo_p.tile([128, 1], F32, tag="rv")
                    nc.vector.reciprocal(rv[:], fi[:, 64:65])
                    nc.vector.tensor_scalar_mul(
                        ob[:, qq, h * 64:(h + 1) * 64], fi[:, 0:64], rv[:])

            nc.sync.dma_start(
                out=out_d[st * ST:(st + 1) * ST, :].rearrange(
                    "(q p) n -> p q n", p=128),
                in_=ob[:])
    return nc


_NC_CACHE = None


def _host_consts():
    i = np.arange(0, D, 2, dtype=np.float64) / D          # 32 pair exponents
    freqs = 1.0 / (10000.0 ** i)                           # (32,)
    ang = np.arange(S, dtype=np.float64)[:, None] * freqs[None, :]  # (S, 32)
    cos = np.cos(ang).astype(np.float32)                   # (S, 32)
    sin = np.sin(ang).astype(np.float32)
    dcol = (np.arange(RC) % D) // 2                        # (320,) pair idx
    cosn = np.ascontiguousarray(cos[:, dcol])              # (S, 320)
    sinn = np.ascontiguousarray(sin[:, dcol])
    kv, qq = np.meshgrid(np.arange(128), np.arange(128), indexing="ij")
    maskinv = (kv > qq).astype(np.uint8)                   # 1 = forbidden
    ident = np.eye(128, dtype=np.float32).astype(ml_dtypes.bfloat16)
    return cosn, sinn, maskinv, ident


def _in_maps(x, Wq, Wk, Wv):
    x = np.asarray(x, dtype=np.float32).reshape(B * S, DIN)
    xt = np.ascontiguousarray(x.T).astype(ml_dtypes.bfloat16)
    Wq = np.asarray(Wq, dtype=np.float32)
    Wk = np.asarray(Wk, dtype=np.float32)
    Wv = np.asarray(Wv, dtype=np.float32)
    cosn, sinn, maskinv, ident = _host_consts()

    in_maps = []
    for k in range(NCORES):
        w_all = np.hstack([
            Wq[:, k * 256:(k + 1) * 256],
            Wk[:, k * 64:(k + 1) * 64],
            Wv[:, k * 64:(k + 1) * 64],
        ]).astype(ml_dtypes.bfloat16)
        in_maps.append({
            "xt": xt, "w": np.ascontiguousarray(w_all),
            "cosn": cosn, "sinn": sinn, "mask": maskinv, "ident": ident,
        })
    return in_maps


def _run(in_maps, **kwargs):
    global _NC_CACHE
    if _NC_CACHE is None:
        _NC_CACHE = build_bass()
        _NC_CACHE.finalize()
    return run_bass_kernel_spmd(_NC_CACHE, in_maps, list(range(NCORES)),
                                **kwargs)


def kernel(x, Wq, Wk, Wv):
    res = _run(_in_maps(x, Wq, Wk, Wv))
    out = np.concatenate([res.results[k]["out"] for k in range(NCORES)], axis=1)
    return out.reshape(B, S, 32 * D)
